# revision 3
# baseline (speedup 1.0000x reference)
"""GCN encoder (7-layer GCNConv) on 8 Trainium2 NeuronCores.

Strategy (node-sharded, SPMD) v2:
  - Nodes permuted into 8 cores x 10 target-tiles of 128 slots (bins balanced
    by per-bucket unique-source counts).  Each core's 1280 nodes split into
    chunk A (groups 0..6, 896 nodes) and chunk B (groups 7..9, 384 nodes).
  - Per layer: z = h @ W computed locally; z is AllGathered in FOUR pieces
    (A/B node-chunk x 2 column halves) so the A-piece collective starts while
    the tail of the previous aggregation still runs (cross-layer pipelining).
  - Aggregation per target group: incoming-edge source rows are DEDUPED per
    (group, chunk-bucket) and fetched with dma_gather from the gathered z;
    segment-sum runs on the TensorEngine via a sparse indicator matrix S
    (S[slot, t] = sum of gcn_norms of edges slot-source -> target t).
  - Dense transform of the NEXT layer is interleaved chunk-by-chunk into the
    aggregation loop (hT transpose-gather + matmuls fire per 256-node chunk),
    so the AllGather of the next layer's z overlaps the aggregation tail.
  - Layer 0 is aggregate-first (width 128), layer 6 transform-first (256).
  - gcn_norm / balancing / dedup / permutation are host-side preprocessing.
"""

import os
import sys
import types

sys.path.insert(0, "/opt/trn_rl_repo")

import numpy as np
import ml_dtypes

NCORES = 8
N = 10000
E = 160000
DIN = 128
DH = 1000
DOUT = 256

TPC = 10  # target tiles (groups) per core
NP_ = TPC * 128  # 1280 node slots per core
NTOT = NCORES * NP_
GA = 7  # groups 0..6 are chunk A
GB = TPC - GA
NAR = GA * 128  # 896 A-rows per core
NBR = GB * 128  # 384 B-rows per core
KT_A = 11  # edge-slot tiles per group, bucket A (deduped sources)
KT_B = 5
KT = KT_A + KT_B  # 16
EPGA = KT_A * 128  # 1408
EPGB = KT_B * 128  # 640
NLP = 1024  # padded hidden width
NQ = 4  # SWDGE queues
NWARM = 48

BF = ml_dtypes.bfloat16

LAYER_NL = [NLP] * 6 + [DOUT]
LAYER_KL = [1] + [8] * 6

_CACHE = {}

LAST_EXEC_NS = None
LAST_TRACE = None


def _install_ntff_shim():
    try:
        import antenv

        if hasattr(antenv, "axon_hooks"):
            return
        from trn_agent_boot.trn_boot import _ntff_profile_via_ctypes

        hook = _ntff_profile_via_ctypes("/opt/axon/libaxon_pjrt.so")
        mod = types.ModuleType("antenv.axon_hooks")
        mod.get_axon_ntff_profile_hook = lambda: hook
        mod.set_axon_ntff_profile_hook = lambda h: None
        sys.modules["antenv.axon_hooks"] = mod
        antenv.axon_hooks = mod
    except Exception:
        pass


def _wrap_idx(idx):
    """[n] int -> [128, n/16] int16 (i -> row i%16, col i//16), 8x replicated."""
    n = idx.shape[0]
    w = np.asarray(idx, np.int16).reshape(n // 16, 16).T
    return np.tile(w, (8, 1))


def _build_bass():
    import concourse.mybir as mybir
    from concourse import bacc, tile

    f32 = mybir.dt.float32
    b16 = mybir.dt.bfloat16
    i16 = mybir.dt.int16
    RG = [list(range(NCORES))]

    nc = bacc.Bacc(
        "TRN2",
        target_bir_lowering=False,
        debug=False,
        num_devices=NCORES,
        num_swdge_queues=NQ,
    )

    xpA_d = nc.dram_tensor("xpA", [NCORES * NAR, DIN], b16, kind="ExternalInput").ap()
    xpB_d = nc.dram_tensor("xpB", [NCORES * NBR, DIN], b16, kind="ExternalInput").ap()
    w_d = [
        nc.dram_tensor(
            f"w{l}", [128, LAYER_KL[l], LAYER_NL[l]], b16, kind="ExternalInput"
        ).ap()
        for l in range(7)
    ]
    bias_d = [
        nc.dram_tensor(f"bias{l}", [128, LAYER_NL[l]], b16, kind="ExternalInput").ap()
        for l in range(7)
    ]
    ones_d = nc.dram_tensor("ones", [128, 128], b16, kind="ExternalInput").ap()
    s_d = nc.dram_tensor("s", [128, TPC, KT, 128], b16, kind="ExternalInput").ap()
    eia_d = nc.dram_tensor(
        "eidxA", [128, TPC, EPGA // 16], i16, kind="ExternalInput"
    ).ap()
    eib_d = nc.dram_tensor(
        "eidxB", [128, TPC, EPGB // 16], i16, kind="ExternalInput"
    ).ap()
    tidx_d = nc.dram_tensor("tidx", [128, 16], i16, kind="ExternalInput").ap()
    out_d = nc.dram_tensor("out", [NP_, DOUT], f32, kind="ExternalOutput").ap()
    out_v = out_d.rearrange("(g p) f -> p g f", p=128)

    qctr = [0]

    def next_q():
        q = qctr[0] % NQ
        qctr[0] += 1
        return q

    with tile.TileContext(nc) as tc:
        with (
            tc.tile_pool(name="const", bufs=1) as cpool,
            tc.tile_pool(name="w", bufs=2) as wpool,
            tc.tile_pool(name="h", bufs=1) as hpool,
            tc.tile_pool(name="ht", bufs=1) as htpool,
            tc.tile_pool(name="z", bufs=3) as zpool,
            tc.tile_pool(name="m", bufs=3) as mpool,
            tc.tile_pool(name="warm", bufs=1, space="PSUM") as warmpool,
            tc.tile_pool(name="o", bufs=2) as opool,
            tc.tile_pool(name="psD", bufs=2, space="PSUM") as psD,
            tc.tile_pool(name="psA", bufs=3, space="PSUM") as psA,
            tc.tile_pool(name="dram", bufs=2, space="DRAM") as dpool,
        ):
            eia_sb = cpool.tile([128, TPC, EPGA // 16], i16)
            nc.sync.dma_start(eia_sb[:], eia_d[:])
            eib_sb = cpool.tile([128, TPC, EPGB // 16], i16)
            nc.sync.dma_start(eib_sb[:], eib_d[:])
            s_sb = cpool.tile([128, TPC, KT, 128], b16)
            nc.sync.dma_start(s_sb[:], s_d[:])
            tidx_sb = cpool.tile([128, 16], i16)
            nc.sync.dma_start(tidx_sb[:], tidx_d[:])
            ones_sb = cpool.tile([128, 128], b16)
            nc.sync.dma_start(ones_sb[:], ones_d[:])
            bias_sb = []
            for l in range(7):
                b_sb = cpool.tile([128, LAYER_NL[l]], b16, name=f"bias_sb{l}")
                nc.sync.dma_start(b_sb[:], bias_d[l][:])
                bias_sb.append(b_sb)

            def gather_group(msgs, srcA, srcB, g, fcw, tagix):
                # A bucket: 1024 + 384 idxs; B bucket: 640 idxs
                nc.gpsimd.dma_gather(
                    msgs[:, 0:8, :],
                    srcA[:],
                    eia_sb[:, g, 0:64],
                    num_idxs=1024,
                    num_idxs_reg=1024,
                    elem_size=fcw,
                    elem_step=fcw,
                    queue_num=next_q(),
                )
                nc.gpsimd.dma_gather(
                    msgs[:, 8:KT_A, :],
                    srcA[:],
                    eia_sb[:, g, 64 : EPGA // 16],
                    num_idxs=EPGA - 1024,
                    num_idxs_reg=EPGA - 1024,
                    elem_size=fcw,
                    elem_step=fcw,
                    queue_num=next_q(),
                )
                nc.gpsimd.dma_gather(
                    msgs[:, KT_A:KT, :],
                    srcB[:],
                    eib_sb[:, g, :],
                    num_idxs=EPGB,
                    num_idxs_reg=EPGB,
                    elem_size=fcw,
                    elem_step=fcw,
                    queue_num=next_q(),
                )

            # per-layer collective bounce + gathered buffers
            def mk_zb(li, fcw):
                zbA = [
                    dpool.tile([NAR, fcw], b16, tag=f"zbA{n}", name=f"zbA{li}_{n}")
                    for n in range(2 if fcw == 512 else 1)
                ]
                zbB = [
                    dpool.tile([NBR, fcw], b16, tag=f"zbB{n}", name=f"zbB{li}_{n}")
                    for n in range(2 if fcw == 512 else 1)
                ]
                zfA = [
                    dpool.tile(
                        [NCORES * NAR, fcw],
                        b16,
                        addr_space="Shared",
                        tag=f"zfA{n}",
                        name=f"zfA{li}_{n}",
                    )
                    for n in range(2 if fcw == 512 else 1)
                ]
                zfB = [
                    dpool.tile(
                        [NCORES * NBR, fcw],
                        b16,
                        addr_space="Shared",
                        tag=f"zfB{n}",
                        name=f"zfB{li}_{n}",
                    )
                    for n in range(2 if fcw == 512 else 1)
                ]
                return zbA, zbB, zfA, zfB

            def ag(zb, zf):
                nc.gpsimd.collective_compute(
                    "AllGather",
                    mybir.AluOpType.bypass,
                    replica_groups=RG,
                    ins=[zb[:].opt()],
                    outs=[zf[:].opt()],
                )

            # dense m-tile (both column halves) of layer `lw` from hT chunk c
            # into bounce buffers; returns last z_sb for warmers.
            def dense_mtile(lw, m, hT_src, w_sb, zbA, zbB, nfc):
                fcw = 512 if LAYER_NL[lw] == NLP else LAYER_NL[lw]
                last = None
                for n2 in range(nfc):
                    zp = psD.tile([128, 512], f32, tag="psD", name=f"zp{lw}_{m}_{n2}")
                    for k in range(LAYER_KL[lw]):
                        nc.tensor.matmul(
                            zp[:, 0:fcw],
                            hT_src[:, k, (m % 2) * 128 : (m % 2) * 128 + 128],
                            w_sb[:, k, n2 * 512 : n2 * 512 + fcw],
                            start=(k == 0),
                            stop=(k == LAYER_KL[lw] - 1),
                        )
                    z_sb = zpool.tile([128, fcw], b16, tag="z", name=f"z{lw}_{m}_{n2}")
                    nc.vector.tensor_copy(z_sb[:], zp[:, 0:fcw])
                    if m < GA:
                        nc.sync.dma_start(
                            zbA[n2][m * 128 : (m + 1) * 128, :], z_sb[:]
                        )
                    else:
                        mm = m - GA
                        nc.sync.dma_start(
                            zbB[n2][mm * 128 : (mm + 1) * 128, :], z_sb[:]
                        )
                    last = z_sb
                return last

            # ---- block 0: conv1 aggregate-first + dense z1 + AG(z1) ----
            aggx_c = [
                cpool.tile([128, 2, DIN], b16, name=f"aggx_c{ci}") for ci in range(5)
            ]
            aggxT_c = [
                cpool.tile([128, 1, 256], b16, name=f"aggxT_c{ci}") for ci in range(5)
            ]
            w0_sb = wpool.tile([128, 1, NLP], b16, tag="w", name="w_sb0")
            nc.sync.dma_start(w0_sb[:], w_d[0][:])
            w1_sb = wpool.tile([128, 8, NLP], b16, tag="w", name="w_sb1")
            nc.sync.dma_start(w1_sb[:], w_d[1][:])
            h1_c = [
                hpool.tile([128, 2, NLP], b16, tag=f"h{ci}", name=f"h0_c{ci}")
                for ci in range(5)
            ]
            hT1_c = [
                htpool.tile([128, 8, 256], b16, tag=f"ht{ci}", name=f"hT0_c{ci}")
                for ci in range(5)
            ]
            zbA1, zbB1, zfA1, zfB1 = mk_zb(1, 512)
            for g in range(TPC):
                msgs0 = mpool.tile([128, KT, DIN], b16, tag="m", name=f"msgs0_{g}")
                gather_group(msgs0, xpA_d, xpB_d, g, DIN, 0)
                ap0 = psA.tile([128, DIN], f32, tag="psA", name=f"ap0_{g}")
                for k in range(KT):
                    nc.tensor.matmul(
                        ap0[:],
                        s_sb[:, g, k, :],
                        msgs0[:, k, :],
                        start=(k == 0),
                        stop=(k == KT - 1),
                    )
                nc.scalar.activation(
                    aggx_c[g // 2][:, g % 2, :],
                    ap0[:],
                    mybir.ActivationFunctionType.Copy,
                )
                if g % 2 == 1:
                    ci = g // 2
                    nc.gpsimd.dma_gather(
                        aggxT_c[ci][:],
                        aggx_c[ci][:],
                        tidx_sb[:],
                        num_idxs=256,
                        num_idxs_reg=256,
                        elem_size=DIN,
                        transpose=True,
                        sbuf_tokens_per_rank=128,
                        sbuf_free_dim_per_rank=DIN * 2,
                        queue_num=next_q(),
                    )
                    # dense conv1: h1 = relu(aggx @ W1 + b1) for m = 2c, 2c+1
                    for m in (2 * ci, 2 * ci + 1):
                        for n2 in range(2):
                            zp0 = psD.tile(
                                [128, 512], f32, tag="psD", name=f"zp0_{m}_{n2}"
                            )
                            nc.tensor.matmul(
                                zp0[:],
                                aggxT_c[ci][
                                    :, 0, (m % 2) * 128 : (m % 2) * 128 + 128
                                ],
                                w0_sb[:, 0, n2 * 512 : n2 * 512 + 512],
                                start=True,
                                stop=False,
                            )
                            nc.tensor.matmul(
                                zp0[:],
                                ones_sb[:],
                                bias_sb[0][:, n2 * 512 : n2 * 512 + 512],
                                start=False,
                                stop=True,
                            )
                            nc.scalar.activation(
                                h1_c[ci][:, m % 2, n2 * 512 : n2 * 512 + 512],
                                zp0[:],
                                mybir.ActivationFunctionType.Relu,
                            )
                    # hT1 chunk + dense z1 = h1 @ W2... (w index 1) + bounce
                    nc.gpsimd.dma_gather(
                        hT1_c[ci][:],
                        h1_c[ci][:],
                        tidx_sb[:],
                        num_idxs=256,
                        num_idxs_reg=256,
                        elem_size=NLP,
                        transpose=True,
                        sbuf_tokens_per_rank=128,
                        sbuf_free_dim_per_rank=NLP * 2,
                        queue_num=next_q(),
                    )
                    for m in (2 * ci, 2 * ci + 1):
                        last_z = dense_mtile(1, m, hT1_c[ci], w1_sb, zbA1, zbB1, 2)
                    if ci == 3:
                        ag(zbA1[0], zfA1[0])
                    if ci == 4:
                        ag(zbB1[0], zfB1[0])
                        ag(zbA1[1], zfA1[1])
                        ag(zbB1[1], zfB1[1])

            # PE warmer through the AG window
            wp = warmpool.tile([128, 512], f32, tag="warm", name="warm0")
            for wi in range(NWARM):
                nc.tensor.matmul(
                    wp[:], ones_sb[:], last_z[:], start=True, stop=True,
                    skip_group_check=True,
                )

            # ---- blocks 1..6: aggregate z_li (+bias, relu), dense z_{li+1} ----
            zfA_cur, zfB_cur = zfA1, zfB1
            hT_prev = hT1_c
            for li in range(1, 7):
                NL = LAYER_NL[li]
                fcw = 512 if NL == NLP else NL
                nch = 2 if NL == NLP else 1
                is_last = li == 6

                if not is_last:
                    wn_sb = wpool.tile(
                        [128, 8, LAYER_NL[li + 1]], b16, tag="w", name=f"w_sb{li+1}"
                    )
                    nc.sync.dma_start(wn_sb[:], w_d[li + 1][:])
                    nfc_next = 2 if LAYER_NL[li + 1] == NLP else 1
                    zbAn, zbBn, zfAn, zfBn = mk_zb(li + 1, LAYER_NL[li + 1] if LAYER_NL[li+1] != NLP else 512)
                    h_c = [
                        hpool.tile(
                            [128, 2, NLP], b16, tag=f"h{ci}", name=f"h{li}_c{ci}"
                        )
                        for ci in range(5)
                    ]
                    hT_c = [
                        htpool.tile(
                            [128, 8, 256], b16, tag=f"ht{ci}", name=f"hT{li}_c{ci}"
                        )
                        for ci in range(5)
                    ]

                for n in range(nch):
                    for g in range(TPC):
                        msgs = mpool.tile(
                            [128, KT, fcw], b16, tag="m", name=f"msgs{li}_{g}_{n}"
                        )
                        gather_group(
                            msgs, zfA_cur[n], zfB_cur[n], g, fcw, li
                        )
                        ap = psA.tile(
                            [128, fcw], f32, tag="psA", name=f"ap{li}_{g}_{n}"
                        )
                        for k in range(KT):
                            nc.tensor.matmul(
                                ap[:],
                                s_sb[:, g, k, :],
                                msgs[:, k, :],
                                start=(k == 0),
                                stop=False,
                            )
                        nc.tensor.matmul(
                            ap[:],
                            ones_sb[:],
                            bias_sb[li][:, n * 512 : n * 512 + fcw],
                            start=False,
                            stop=True,
                        )
                        if is_last:
                            o_sb = opool.tile([128, DOUT], f32, tag="o", name=f"o{g}")
                            nc.scalar.activation(
                                o_sb[:], ap[:], mybir.ActivationFunctionType.Copy
                            )
                            nc.sync.dma_start(out_v[:, g, :], o_sb[:])
                            continue
                        nc.scalar.activation(
                            h_c[g // 2][:, g % 2, n * 512 : n * 512 + fcw],
                            ap[:],
                            mybir.ActivationFunctionType.Relu,
                        )
                        if n == nch - 1 and g % 2 == 1:
                            ci = g // 2
                            nc.gpsimd.dma_gather(
                                hT_c[ci][:],
                                h_c[ci][:],
                                tidx_sb[:],
                                num_idxs=256,
                                num_idxs_reg=256,
                                elem_size=NLP,
                                transpose=True,
                                sbuf_tokens_per_rank=128,
                                sbuf_free_dim_per_rank=NLP * 2,
                                queue_num=next_q(),
                            )
                            for m in (2 * ci, 2 * ci + 1):
                                last_z = dense_mtile(
                                    li + 1, m, hT_c[ci], wn_sb, zbAn, zbBn, nfc_next
                                )
                            if ci == 3:
                                ag(zbAn[0], zfAn[0])
                            if ci == 4:
                                ag(zbBn[0], zfBn[0])
                                if nfc_next == 2:
                                    ag(zbAn[1], zfAn[1])
                                    ag(zbBn[1], zfBn[1])
                if not is_last:
                    fw = 512 if LAYER_NL[li + 1] == NLP else LAYER_NL[li + 1]
                    wp = warmpool.tile([128, 512], f32, tag="warm", name=f"warm{li}")
                    for wi in range(NWARM):
                        nc.tensor.matmul(
                            wp[:, 0:fw], ones_sb[:], last_z[:], start=True, stop=True,
                            skip_group_check=True,
                        )
                    zfA_cur, zfB_cur = zfAn, zfBn
                    hT_prev = hT_c

    # Align each gather's SWDGE queue with its Tile-assigned DMASW sem lane
    from concourse.tile_sem_assignment import PROC_NAME_TO_IDX

    lane_to_q = {PROC_NAME_TO_IDX[f"DMASW{i}"]: i % NQ for i in range(8)}
    for bb in nc.main_func.blocks:
        for inst in bb.instructions:
            if isinstance(inst, mybir.InstDMAGatherAnt):
                proc = getattr(inst, "bass_scheduled_proc", None)
                if proc in lane_to_q:
                    inst.queue_num = lane_to_q[proc]

    nc.compile()
    return nc


def _preprocess(x, edge_index, edge_weight):
    """gcn_norm + unique-aware balancing + per-(group,bucket) source dedup."""
    ei = np.asarray(edge_index)
    row = np.concatenate([ei[0], np.arange(N)]).astype(np.int64)
    col = np.concatenate([ei[1], np.arange(N)]).astype(np.int64)
    w = np.concatenate(
        [np.asarray(edge_weight, np.float64), np.ones(N, np.float64)]
    )
    deg = np.zeros(N, np.float64)
    np.add.at(deg, col, w)
    dis = np.where(deg > 0, 1.0 / np.sqrt(deg), 0.0)
    norm = (dis[row] * w * dis[col]).astype(np.float32)

    NB = NCORES * TPC
    order_e = np.argsort(col, kind="stable")
    col_s, row_s = col[order_e], row[order_e]
    w_s = norm[order_e]
    starts = np.searchsorted(col_s, np.arange(N + 1))
    indeg = np.diff(starts)
    order = np.argsort(-indeg, kind="stable")

    # pass 1: edge-count balance (defines provisional A/B membership)
    load = np.zeros(NB)
    cnt = np.zeros(NB, np.int64)
    binof = np.empty(N, np.int64)
    for v in order:
        feas = np.flatnonzero(cnt < 128)
        b = feas[np.argmin(load[feas])]
        binof[v] = b
        cnt[b] += 1
        load[b] += indeg[v]

    def balance(grpA):
        SETS = np.zeros((NB, N), bool)
        uA = np.zeros(NB, np.int64)
        uB = np.zeros(NB, np.int64)
        cnt = np.zeros(NB, np.int64)
        bo = np.empty(N, np.int64)
        so = np.empty(N, np.int64)
        for v in order:
            s = row_s[starts[v] : starts[v + 1]]
            isA = grpA[s]
            sub = SETS[:, s]
            newA = (~sub[:, isA]).sum(1)
            newB = (~sub[:, ~isA]).sum(1)
            feas = (cnt < 128) & (uA + newA <= EPGA) & (uB + newB <= EPGB)
            if not feas.any():
                return None
            metric = np.maximum((uA + newA) / EPGA, (uB + newB) / EPGB) + 1e-9 * cnt
            metric[~feas] = 9e9
            b = int(np.argmin(metric))
            bo[v] = b
            so[v] = cnt[b]
            cnt[b] += 1
            uA[b] += newA[b]
            uB[b] += newB[b]
            SETS[b, s] = True
        return bo, so

    # pass 2 (+retries): unique-aware balance under per-bucket caps, verified
    # against the TRUE post-assignment A/B membership.
    slotof = None
    for _ in range(4):
        grpA = (binof % TPC) < GA
        r = balance(grpA)
        assert r is not None, "bucket balance infeasible"
        bo, so = r
        grpA2 = (bo % TPC) < GA
        srcA_e = grpA2[row]
        ebin = bo[col]
        okA = np.ones(NB, bool)
        uAmax = uBmax = 0
        for b in range(NB):
            m = ebin == b
            ua = len(np.unique(row[m][srcA_e[m]]))
            ub = len(np.unique(row[m][~srcA_e[m]]))
            uAmax = max(uAmax, ua)
            uBmax = max(uBmax, ub)
        binof, slotof = bo, so
        if uAmax <= EPGA and uBmax <= EPGB:
            break
    else:
        raise AssertionError(f"balance failed: uA {uAmax} uB {uBmax}")

    core = binof // TPC
    grp = binof % TPC
    pid = core * NP_ + grp * 128 + slotof
    isA_n = grp < GA
    arow = np.where(
        isA_n, core * NAR + grp * 128 + slotof, core * NBR + (grp - GA) * 128 + slotof
    )

    # per-(bin,bucket) dedup: assign slots to unique sources, accumulate S
    S = np.zeros((NCORES, 128, TPC, KT, 128), np.float32)
    IDXA = np.zeros((NCORES, TPC, EPGA), np.int64)
    IDXB = np.zeros((NCORES, TPC, EPGB), np.int64)
    ebin = binof[col]
    esrcA = isA_n[row]
    for b in range(NB):
        ec, eg = b // TPC, b % TPC
        m = ebin == b
        srcs = row[m]
        tgt = slotof[col[m]]
        nm = norm[m]
        sA = esrcA[m]
        for bucket, ktoff, idxarr, cap in (
            (sA, 0, IDXA, EPGA),
            (~sA, KT_A, IDXB, EPGB),
        ):
            bs = srcs[bucket]
            bt = tgt[bucket]
            bn = nm[bucket]
            uniq, inv = np.unique(bs, return_inverse=True)
            assert len(uniq) <= cap
            ek = ktoff + inv // 128
            ep = inv % 128
            np.add.at(S, (ec, ep, eg, ek, bt), bn)
            idxarr[ec, eg, : len(uniq)] = arow[uniq]
    return pid, S, IDXA, IDXB


def kernel(x, edge_index, edge_weight, W1, b1, Wmid, bmid, W7, b7):
    global LAST_EXEC_NS, LAST_TRACE
    trace = os.environ.get("GCN_TRACE") == "1"
    if trace:
        _install_ntff_shim()

    from concourse import bass_utils

    x = np.asarray(x, np.float32)
    pid, S, IDXA, IDXB = _preprocess(x, edge_index, edge_weight)

    # x in A/B bucket row order, bf16, empty slots zero; replicated per core
    grp = (pid % NP_) // 128
    slot = pid % 128
    core = pid // NP_
    isA_n = grp < GA
    xpA = np.zeros((NCORES * NAR, DIN), np.float32)
    xpB = np.zeros((NCORES * NBR, DIN), np.float32)
    arowA = core * NAR + grp * 128 + slot
    arowB = core * NBR + (grp - GA) * 128 + slot
    xpA[arowA[isA_n]] = x[isA_n]
    xpB[arowB[~isA_n]] = x[~isA_n]
    xpA = xpA.astype(BF)
    xpB = xpB.astype(BF)

    def kstripe(W, KL, NL):
        Wp = np.zeros((KL * 128, NL), np.float32)
        Wp[: W.shape[0], : W.shape[1]] = np.asarray(W, np.float32)
        return Wp.reshape(KL, 128, NL).transpose(1, 0, 2).astype(BF)

    Ws = [kstripe(np.asarray(W1), 1, NLP)]
    for i in range(5):
        Ws.append(kstripe(np.asarray(Wmid)[i], 8, NLP))
    Ws.append(kstripe(np.asarray(W7), 8, DOUT))
    bs = []
    for i, b in enumerate([b1] + [np.asarray(bmid)[i] for i in range(5)] + [b7]):
        NL = LAYER_NL[i]
        bp = np.zeros(NL, np.float32)
        bp[: b.shape[0]] = np.asarray(b, np.float32)
        bs.append(np.broadcast_to(bp.astype(BF), (128, NL)).copy())

    ones = np.full((128, 128), 1.0 / 128.0, np.float32).astype(BF)
    tidx = _wrap_idx(np.arange(256))

    if "nc" not in _CACHE:
        _CACHE["nc"] = _build_bass()
    nc = _CACHE["nc"]

    in_maps = []
    for c in range(NCORES):
        eidxA_c = np.stack([_wrap_idx(IDXA[c, g]) for g in range(TPC)], axis=1)
        eidxB_c = np.stack([_wrap_idx(IDXB[c, g]) for g in range(TPC)], axis=1)
        m = {
            "xpA": xpA,
            "xpB": xpB,
            "ones": ones,
            "s": np.ascontiguousarray(S[c].astype(BF)),
            "eidxA": np.ascontiguousarray(eidxA_c),
            "eidxB": np.ascontiguousarray(eidxB_c),
            "tidx": tidx,
        }
        for l in range(7):
            m[f"w{l}"] = Ws[l]
            m[f"bias{l}"] = bs[l]
        in_maps.append(m)

    res = bass_utils.run_bass_kernel_spmd(
        nc, in_maps, core_ids=list(range(NCORES)), trace=trace
    )
    if trace:
        LAST_EXEC_NS = res.exec_time_ns
        LAST_TRACE = res.profile_json
        print(f"HW exec time: {res.exec_time_ns} ns")
        if res.instructions_and_trace is not None:
            print(f"trace: {res.instructions_and_trace[1]}")

    percore = np.stack([res.results[c]["out"] for c in range(NCORES)])
    out_full = percore[pid // NP_, pid % NP_]
    return out_full
